# revision 32
# baseline (speedup 1.0000x reference)
"""Trainium2 Bass kernel for nn_ContextEncoder.

Pipeline (per sample b): feature transform tanh(X @ W_t.T + b_t), a
"bidirectional" LSTM where both directions run forward (matching the
reference), attention pooling against the last hidden state, and a
context norm over the flattened (d, 2h) vector.

Sharding: data-parallel over b (16 samples -> 2 per core on 8 cores).
Each core runs 128 independent sequences (2 b x 64 d) of length T=128.

Layout: gates/hidden live on partitions and batch j on the free axis, so
the recurrence's serial chain per direction is just
  MM(whh chunks) -> sigmoid -> (uh, c2, c'=uh+c2) -> tanh(2c') -> h
with no transposes on the chain (the attention-layout transposes and
their PSUM evacuation run off-chain). The two direction chains are
software-pipelined half a step apart, the c-update tracks c' = c/2 so it
needs only cheap fused/plain DVE ops (the 2x rides the tanh input
scale), filler matmuls on constants keep the PE's HAM clock gate warm,
the feature transform runs in f16, and the tail's 128 weighted copies
are split between the vector and scalar engines.
"""

import sys

for _p in ("/opt/trn_rl_repo", "/root/.axon_site/_ro/trn_rl_repo"):
    if _p not in sys.path:
        sys.path.append(_p)

import numpy as np

import concourse.bass as bass
import concourse.bacc as bacc
import concourse.tile as tile
from concourse import mybir
from concourse.bass_utils import run_bass_kernel_spmd

F16NP = np.float16
F32 = mybir.dt.float32
F16 = mybir.dt.float16
AF = mybir.ActivationFunctionType
ALU = mybir.AluOpType

B, T, D, NF = 16, 128, 64, 32
TS, H = 64, 128
NCORES = 8
BLOC = B // NCORES          # 2 samples per core
J = BLOC * D                # 128 sequences per core
R = J * T                   # 16384 (t, j) columns
G4 = 4 * H                  # 512 gates per direction
PERM = (0, 1, 3, 2)         # torch gate order (i,f,g,o) -> (i,f,o,g)
NORM_N = D * 2 * H          # 16384 context-norm elements per sample
FCH = 1024                  # feature-transform chunk (columns)

GP_C2 = False               # GPSIMD hop costs more latency than it saves
EVAC_DMA = False            # PSUM has no DMA read port; evacuate on DVE


def emit(tc, ins, outs):
    nc = tc.nc
    with tc.tile_pool(name="consts", bufs=1) as consts:
        wtt = consts.tile([NF, TS], F16)
        nc.sync.dma_start(wtt, ins["WTT"])
        bt = consts.tile([TS, 1], F32)
        nc.sync.dma_start(bt, ins["BT"])
        wiht = consts.tile([TS + 1, 2, G4], F16)
        nc.sync.dma_start(wiht, ins["WIHT"])
        whht = consts.tile([H, 2, G4], F16)
        nc.sync.dma_start(whht, ins["WHHT"])
        idn = consts.tile([H, H], F16)
        nc.sync.dma_start(idn, ins["IDN"])
        # h history in attention layout, filled via PE transposes + DMA
        ht = consts.tile([J, T, 2, H], F16)

        with (
            tc.tile_pool(name="xsp", bufs=1) as xsp,
            tc.tile_pool(name="hstp", bufs=1) as hstp,
            tc.tile_pool(name="sgp", bufs=2) as sgp,
            tc.tile_pool(name="cp", bufs=2) as cp,
            tc.tile_pool(name="small", bufs=3) as small,
            tc.tile_pool(name="pgf", bufs=1, space="PSUM") as pgf,
            tc.tile_pool(name="pgb", bufs=1, space="PSUM") as pgb,
            tc.tile_pool(name="trp", bufs=2, space="PSUM") as trp,
        ):
            _recurrence(tc, ins, xsp, hstp, sgp, cp, small,
                        (pgf, pgb), trp, wtt, bt, wiht, whht, idn, ht)
        _tail(tc, ins, outs, ht)


def _recurrence(tc, ins, xsp, hstp, sgp, cp, small, pgp, trp,
                wtt, bt, wiht, whht, idn, ht):
    nc = tc.nc
    # ---- feature transform: xs2[0:64, (t,j)] = tanh(Wt @ X.T + bt) ----
    xs2 = xsp.tile([TS + 1, R], F16)
    nc.sync.dma_start(xs2[TS : TS + 1, :], ins["ONES"])
    with (
        tc.tile_pool(name="xtp", bufs=4) as xtp,
        tc.tile_pool(name="tfp", bufs=1, space="PSUM") as tfp,
    ):
        for cc in range(R // FCH):
            sl = slice(cc * FCH, (cc + 1) * FCH)
            xt = xtp.tile([NF, FCH], F16, tag="xt")
            nc.sync.dma_start(xt, ins["XT"][:, sl])
            pz = tfp.tile([TS, 2, 512], F32, tag="pz")
            nc.tensor.matmul(pz[:, 0, :], lhsT=wtt, rhs=xt[:, 0:512],
                             start=True, stop=True)
            nc.tensor.matmul(pz[:, 1, :], lhsT=wtt, rhs=xt[:, 512:FCH],
                             start=True, stop=True)
            nc.scalar.activation(
                out=xs2[0:TS, sl],
                in_=pz.rearrange("p a b -> p (a b)"),
                func=AF.Tanh, bias=bt, scale=1.0,
            )

    # ---- recurrence state ----
    # junk matmuls on constants fill PE stall windows so the HAM clock
    # gate keeps the PE at 2.4 GHz (cold MMs run at 1.2 GHz otherwise)
    jpool_cm = tc.tile_pool(name="junkp", bufs=1, space="PSUM")
    jpool = jpool_cm.__enter__()
    junk = jpool.tile([H, 512], F32, name="junk")

    def warm(n):
        for _ in range(n):
            nc.tensor.matmul(junk, lhsT=idn, rhs=whht[:, 0, :],
                             start=True, stop=True, skip_group_check=True)

    hsl = [hstp.tile([H, 4, J], F16, name=f"hs{d}") for d in range(2)]
    pg = [None, None]
    sg = [None, None]
    cn = [None, None]
    tcv = [None, None]
    ptr_box = [None]

    def mm_wih(t):
        # open the 2-step PSUM accumulation groups for both dirs
        rhs = xs2[:, t * J : (t + 2) * J]
        for d in range(2):
            pg[d] = pgp[d].tile([H, 4, 2 * J], F32, tag=f"pg{d}",
                                name=f"pg{d}")
            for c in range(4):
                nc.tensor.matmul(pg[d][:, c, :],
                                 lhsT=wiht[:, d, c * H : (c + 1) * H],
                                 rhs=rhs, start=(c % 2 == 0), stop=False)

    def mm_whh(d, t):
        u0 = (t % 2) * J
        rhs = hsl[d][:, (t - 1) % 4, :]
        for c in range(4):
            nc.tensor.matmul(pg[d][:, c, u0 : u0 + J],
                             lhsT=whht[:, d, c * H : (c + 1) * H],
                             rhs=rhs, start=False,
                             stop=(t % 2 == 1 and c % 2 == 1))

    def sigma(d, t):
        u0 = (t % 2) * J
        sg[d] = sgp.tile([H, 4, J], F16, tag=f"sg{d}", name=f"sg{d}")
        nc.scalar.activation(out=sg[d], in_=pg[d][:, :, u0 : u0 + J],
                             func=AF.Sigmoid)

    def dvec(d, t):
        # track c' = c/2:  c'_t = sigma(f)*c'_{t-1} + uh,
        # uh = (sig(2g)-0.5)*sigma(i) = tanh(g)*sigma(i)/2; tanh(c)
        # recovers the 2x via the activation's free input scale.
        uh = small.tile([H, J], F16, tag=f"uh{d}", name=f"uh{d}")
        nc.vector.scalar_tensor_tensor(
            out=uh, in0=sg[d][:, 3, :], scalar=0.5, in1=sg[d][:, 0, :],
            op0=ALU.subtract, op1=ALU.mult)
        c_new = cp.tile([H, J], F16, tag=f"c{d}", name=f"cn{d}")
        if t > 0:
            c2 = small.tile([H, J], F16, tag=f"c2{d}", name=f"c2{d}")
            eng = nc.gpsimd if GP_C2 else nc.vector
            eng.tensor_mul(c2, sg[d][:, 1, :], cn[d])
            nc.vector.tensor_add(c_new, uh, c2)
        else:
            nc.vector.tensor_copy(c_new, uh)
        cn[d] = c_new

    def tailc(d, t):
        tcv[d] = small.tile([H, J], F16, tag=f"tc{d}", name=f"tc{d}")
        nc.scalar.activation(out=tcv[d], in_=cn[d], func=AF.Tanh, scale=2.0)
        nc.vector.tensor_mul(hsl[d][:, t % 4, :], sg[d][:, 2, :], tcv[d])

    def transp(d, t):
        # off-chain: h block -> [J, H] in PSUM; DMA pairs out to ht
        if d == 0 and t % 2 == 0:
            ptr_box[0] = trp.tile([J, 2, 2, H], F16, tag="tr", name="ptr")
        ptr = ptr_box[0]
        nc.tensor.transpose(ptr[:, t % 2, d, :], hsl[d][:, t % 4, :], idn)
        if d == 1 and t % 2 == 1:
            t0 = t - 1
            if EVAC_DMA:
                nc.sync.dma_start(ht[:, t0 : t0 + 2, :, :], ptr)
            else:
                nc.vector.tensor_copy(
                    ht[:, t0 : t0 + 2, :, :].rearrange("j a d h -> j (a d h)"),
                    ptr.rearrange("j a d h -> j (a d h)"))

    # ---- software-pipelined emission: dir 1 trails dir 0 by half a step
    for t in range(T):
        if t % 2 == 0:
            mm_wih(t)
        if t > 0:
            warm(3)
            mm_whh(0, t)
            dvec(1, t - 1)
        sigma(0, t)
        if t > 0:
            tailc(1, t - 1)
            transp(1, t - 1)
            warm(3)
            mm_whh(1, t)
        dvec(0, t)
        sigma(1, t)
        tailc(0, t)
        transp(0, t)
    dvec(1, T - 1)
    tailc(1, T - 1)
    transp(1, T - 1)
    # keep the junk psum alive so the filler matmuls aren't dead code
    jrd = small.tile([H, 1], F32, name="jrd")
    nc.vector.tensor_copy(jrd, junk[:, 0:1])
    jpool_cm.__exit__(None, None, None)


def _tail(tc, ins, outs, ht):
    # ---- tail: attention pooling + context norm ----
    nc = tc.nc
    OUT = outs["OUT"]
    ht4 = ht.rearrange("j t d h -> j t (d h)")
    with (
        tc.tile_pool(name="tailp", bufs=1) as tailp,
        tc.tile_pool(name="tailps", bufs=1, space="PSUM") as tailps,
    ):
        # hoist the norm-constant loads so they never gate the final ops
        sel = tailp.tile([J, BLOC], F32)
        nc.sync.dma_start(sel, ins["SEL"])
        selt = tailp.tile([BLOC, J], F32)
        nc.sync.dma_start(selt, ins["SELT"])
        dwt = tailp.tile([J, 2 * H], F32)
        nc.sync.dma_start(dwt[0:D, :], ins["DW"])
        nc.sync.dma_start(dwt[D:J, :], ins["DW"])
        dbt = tailp.tile([J, 2 * H], F32)
        nc.sync.dma_start(dbt[0:D, :], ins["DB"])
        nc.sync.dma_start(dbt[D:J, :], ins["DB"])

        htj = ht4[:, T - 1, :]  # [J, 2H] last hidden state
        htj_b = bass.AP(
            tensor=htj.tensor, offset=htj.offset,
            ap=[list(htj.ap[0]), [0, T], list(htj.ap[-1])],
        )
        prod = tailp.tile([J, T, 2 * H], F16)
        nc.vector.tensor_mul(prod, ht4, htj_b)
        # pairwise-tree sum over p: f16 levels ping-pong {pp0, prod}, then fp32
        pp0 = tailp.tile([J, T, 128], F16)
        nc.vector.tensor_add(pp0, prod[:, :, 0:128], prod[:, :, 128:256])
        nc.vector.tensor_add(prod[:, :, 0:64], pp0[:, :, 0:64], pp0[:, :, 64:128])
        nc.vector.tensor_add(pp0[:, :, 0:32], prod[:, :, 0:32], prod[:, :, 32:64])
        ltrf = tailp.tile([J, T, 16], F32)
        nc.vector.tensor_add(ltrf, pp0[:, :, 0:16], pp0[:, :, 16:32])
        ltr1 = tailp.tile([J, T], F32)
        nc.vector.tensor_reduce(ltr1, ltrf, axis=mybir.AxisListType.X,
                                op=ALU.add)
        logits = ltr1[:, :]
        mx = tailp.tile([J, 1], F32)
        nc.vector.tensor_reduce(mx, logits, axis=mybir.AxisListType.X, op=ALU.max)
        mxn = tailp.tile([J, 1], F32)
        nc.vector.tensor_scalar_mul(mxn, mx, -1.0)
        ew = tailp.tile([J, T], F32)
        dsum = tailp.tile([J, 1], F32)
        nc.scalar.activation(out=ew, in_=logits, func=AF.Exp, bias=mxn,
                             scale=1.0, accum_out=dsum)
        rd = tailp.tile([J, 1], F32)
        nc.vector.reciprocal(rd, dsum)
        nc.vector.tensor_scalar_mul(ew, ew, rd)  # softmax weights in place
        prod2 = tailp.tile([J, T, 2 * H], F16, tag="prod")  # reuse slab
        # ew[j,t] is a per-partition scalar for fixed t; spread the 128
        # weighted copies across DVE / ACT / GPSIMD
        for tt in range(T):
            ws = ew[:, tt : tt + 1]
            if tt < 86:
                nc.vector.tensor_scalar_mul(prod2[:, tt, :], ht4[:, tt, :], ws)
            else:
                nc.scalar.mul(prod2[:, tt, :], ht4[:, tt, :], ws)
        # pairwise-tree sum over t
        qq = pp0.rearrange("j a b -> j (a b)").rearrange(
            "j (a b) -> j a b", a=64)
        nc.vector.tensor_add(qq, prod2[:, 0:64, :], prod2[:, 64:128, :])
        nc.vector.tensor_add(prod2[:, 0:32, :], qq[:, 0:32, :], qq[:, 32:64, :])
        nc.vector.tensor_add(qq[:, 0:16, :], prod2[:, 0:16, :],
                             prod2[:, 16:32, :])
        # write the 8 remaining t-slices transposed to [J, 2H, 8] so the
        # final sum over t is one innermost-axis tensor_reduce
        ptrf = tailp.tile([J, 2 * H, 8], F32)
        ptrv = ptrf.rearrange("j p w -> j w p")
        nc.vector.tensor_add(ptrv, qq[:, 0:8, :], qq[:, 8:16, :])
        pooled = tailp.tile([J, 2 * H], F32)
        nc.vector.tensor_reduce(pooled, ptrf, axis=mybir.AxisListType.X,
                                op=ALU.add)

        # context norm across each sample's (d, 2h) block
        pooled2 = tailp.tile([J, 2 * H], F32)
        nc.scalar.activation(out=pooled2, in_=pooled, func=AF.Square)
        pstat = tailps.tile([BLOC, 2 * G4], F32, tag="stats")
        nc.tensor.matmul(pstat[:, 0 : 2 * H], lhsT=sel, rhs=pooled,
                         start=True, stop=False)
        nc.tensor.matmul(pstat[:, 2 * H : 4 * H], lhsT=sel, rhs=pooled2,
                         start=False, stop=True)
        s1 = tailp.tile([BLOC, 1], F32)
        nc.vector.tensor_reduce(s1, pstat[:, 0 : 2 * H],
                                axis=mybir.AxisListType.X, op=ALU.add)
        s2 = tailp.tile([BLOC, 1], F32)
        nc.vector.tensor_reduce(s2, pstat[:, 2 * H : 4 * H],
                                axis=mybir.AxisListType.X, op=ALU.add)
        stats2 = tailp.tile([BLOC, 2], F32)
        nc.scalar.mul(stats2[:, 0:1], s1, 1.0 / NORM_N)      # mean
        q = tailp.tile([BLOC, 1], F32)
        nc.vector.tensor_mul(q, s1, stats2[:, 0:1])          # sum*mean
        v = tailp.tile([BLOC, 1], F32)
        nc.vector.tensor_tensor(v, s2, q, op=ALU.subtract)
        sd = tailp.tile([BLOC, 1], F32)
        nc.scalar.activation(out=sd, in_=v, func=AF.Sqrt,
                             scale=1.0 / (NORM_N - 1))
        nc.vector.reciprocal(stats2[:, 1:2], sd)             # rstd
        pmb = tailps.tile([J, 2], F32, tag="mb")
        nc.tensor.matmul(pmb, lhsT=selt, rhs=stats2, start=True, stop=True)
        mb = tailp.tile([J, 2], F32)
        nc.vector.tensor_copy(mb, pmb)
        t1 = tailp.tile([J, 2 * H], F32)
        nc.vector.tensor_scalar(t1, pooled, mb[:, 0:1], mb[:, 1:2],
                                op0=ALU.subtract, op1=ALU.mult)
        t2 = tailp.tile([J, 2 * H], F32)
        nc.vector.tensor_mul(t2, t1, dwt)
        t3 = tailp.tile([J, 2 * H], F32)
        nc.vector.tensor_add(t3, t2, dbt)
        nc.sync.dma_start(OUT, t3)


def build_program():
    nc = bacc.Bacc("TRN2", target_bir_lowering=False, debug=False)
    ins = {
        "XT": nc.dram_tensor("XT", [NF, R], F16, kind="ExternalInput").ap(),
        "WTT": nc.dram_tensor("WTT", [NF, TS], F16, kind="ExternalInput").ap(),
        "BT": nc.dram_tensor("BT", [TS, 1], F32, kind="ExternalInput").ap(),
        "WIHT": nc.dram_tensor("WIHT", [TS + 1, 2, G4], F16, kind="ExternalInput").ap(),
        "WHHT": nc.dram_tensor("WHHT", [H, 2, G4], F16, kind="ExternalInput").ap(),
        "ONES": nc.dram_tensor("ONES", [1, R], F16, kind="ExternalInput").ap(),
        "IDN": nc.dram_tensor("IDN", [H, H], F16, kind="ExternalInput").ap(),
        "DW": nc.dram_tensor("DW", [D, 2 * H], F32, kind="ExternalInput").ap(),
        "DB": nc.dram_tensor("DB", [D, 2 * H], F32, kind="ExternalInput").ap(),
        "SEL": nc.dram_tensor("SEL", [J, BLOC], F32, kind="ExternalInput").ap(),
        "SELT": nc.dram_tensor("SELT", [BLOC, J], F32, kind="ExternalInput").ap(),
    }
    outs = {
        "OUT": nc.dram_tensor("OUT", [J, 2 * H], F32, kind="ExternalOutput").ap(),
    }
    with tile.TileContext(nc) as tc:
        emit(tc, ins, outs)
    nc.compile()
    return nc


def _prep_dir(Wih, Whh, bih, bhh):
    # gate order (i,f,o,g); the g block is pre-scaled by 2 so the kernel can
    # evaluate tanh(g) as 2*sigmoid(2g)-1 inside the fused sigmoid op
    wihT = Wih.T.reshape(TS, 4, H)[:, PERM, :].reshape(TS, G4).copy()
    biasr = (bih + bhh).reshape(4, H)[PERM, :].reshape(G4).copy()
    wihT[:, 3 * H :] *= 2.0
    biasr[3 * H :] *= 2.0
    wih65 = np.concatenate([wihT, biasr[None, :]], axis=0).astype(F16NP)
    whhT = Whh.T.reshape(H, 4, H)[:, PERM, :].reshape(H, G4).copy()
    whhT[:, 3 * H :] *= 2.0
    whhT = whhT.astype(F16NP)
    return wih65, whhT


def prep_inputs(X, W_t, b_t, Wih_f, Whh_f, bih_f, bhh_f,
                Wih_b, Whh_b, bih_b, bhh_b, diag_w, diag_b):
    wih_f, whh_f = _prep_dir(Wih_f, Whh_f, bih_f, bhh_f)
    wih_b, whh_b = _prep_dir(Wih_b, Whh_b, bih_b, bhh_b)
    shared = {
        "WTT": np.ascontiguousarray(W_t.T.astype(F16NP)),
        "BT": np.ascontiguousarray(b_t.reshape(TS, 1), dtype=np.float32),
        "WIHT": np.ascontiguousarray(np.stack([wih_f, wih_b], axis=1)),
        "WHHT": np.ascontiguousarray(np.stack([whh_f, whh_b], axis=1)),
        "ONES": np.ones((1, R), dtype=F16NP),
        "IDN": np.eye(H, dtype=F16NP),
        "SEL": np.kron(np.eye(BLOC, dtype=np.float32), np.ones((D, 1), np.float32)),
        "SELT": np.kron(np.eye(BLOC, dtype=np.float32), np.ones((1, D), np.float32)),
        "DW": np.ascontiguousarray(diag_w.reshape(D, 2 * H), dtype=np.float32),
        "DB": np.ascontiguousarray(diag_b.reshape(D, 2 * H), dtype=np.float32),
    }
    in_maps = []
    for i in range(NCORES):
        xt = np.ascontiguousarray(
            X[i * BLOC : (i + 1) * BLOC].transpose(3, 1, 0, 2).reshape(NF, R)
        ).astype(F16NP)
        m = {"XT": xt}
        m.update(shared)
        in_maps.append(m)
    return in_maps


def kernel(**inputs):
    inputs = {k: np.asarray(v, dtype=np.float32) for k, v in inputs.items()}
    in_maps = prep_inputs(**inputs)
    nc = build_program()
    res = run_bass_kernel_spmd(nc, in_maps, list(range(NCORES)))
    out = np.concatenate(
        [res.results[i]["OUT"].reshape(BLOC, D, 2 * H) for i in range(NCORES)],
        axis=0,
    )
    return np.ascontiguousarray(out, dtype=np.float32)


if __name__ == "__main__":
    nc = build_program()
    print("program built ok")
